# revision 11
# baseline (speedup 1.0000x reference)
"""Trainium2 Bass kernel for nn_PolNetLSTM (masked LSTM policy net).

Strategy (8 NeuronCores):
 - gx = relu(xs@W_in.T+b_in)@W_ih.T + b  is T-sharded across the 8 cores
   (big parallel matmuls), then AllGathered (one collective).
 - The sequential LSTM recurrence is replicated on every core (per-step
   cross-core collectives are far too expensive: >4.6us floor/call).
   Layout: partitions = (C-half, batch) [128], free = gates [1024] so the
   per-step matmul streams W_hh with h.T as the stationary operand, all
   elementwise ops run on 128 partitions, and the batch mask is a
   per-partition scalar fused into scalar_tensor_tensor ops.
 - means = tanh(h@W_mean.T) computed on the fly (batched every 4 steps),
   off the critical path.  Final h/c DMAd out at t=T-1.
 - Host does the trivial unpacking/transposes + log_std broadcast.
"""

import numpy as np
import ml_dtypes

import concourse.bass as bass
import concourse.bacc as bacc
import concourse.mybir as mybir
import concourse.tile as tile
from concourse.bass_utils import run_bass_kernel_spmd

F32 = mybir.dt.float32
BF16 = mybir.dt.bfloat16
AF = mybir.ActivationFunctionType
OP = mybir.AluOpType

T, B, OB, AC = 512, 64, 128, 32
H, C = 1024, 512
NCORES = 8

# dtype config: recurrence matmul dtype and gx storage dtype
REC_BF16 = True
GX_BF16 = True

_cache = {}


def _np_dt(dt):
    return ml_dtypes.bfloat16 if dt == BF16 else np.float32


def build(t_steps=T, rec_bf16=REC_BF16, gx_bf16=GX_BF16, mode="full"):
    RDT = BF16 if rec_bf16 else F32
    GDT = BF16 if gx_bf16 else F32
    TS = t_steps // NCORES          # local T slice for gx compute
    ROWS = TS * B                   # local gx rows
    RCH = ROWS // 128               # row chunks of 128 (2 steps each)

    nc = bacc.Bacc("TRN2", target_bir_lowering=False, debug=False,
                   num_devices=NCORES)

    # ---------------- parameters ----------------
    xsT = nc.declare_dram_parameter("xsT", [OB, ROWS], BF16, isOutput=False)
    w_inT = nc.declare_dram_parameter("w_inT", [OB, H], BF16, isOutput=False)
    b_in_col = nc.declare_dram_parameter("b_in_col", [128, H // 128], F32, isOutput=False)
    w_ihT = nc.declare_dram_parameter("w_ihT", [H, 4 * C], BF16, isOutput=False)
    bias_row = nc.declare_dram_parameter("bias_row", [1, 4 * C], F32, isOutput=False)
    w_rec = nc.declare_dram_parameter("w_rec", [C, 4 * C], RDT, isOutput=False)
    w_meanT = nc.declare_dram_parameter("w_meanT", [128, 4 * AC], RDT, isOutput=False)
    b_mean_p = nc.declare_dram_parameter("b_mean_p", [AC, 1], F32, isOutput=False)
    s_dup = nc.declare_dram_parameter("s_dup", [128, t_steps + 1], F32, isOutput=False)
    s_flat = nc.declare_dram_parameter("s_flat", [1, (t_steps + 1) * B], F32, isOutput=False)
    hT0 = nc.declare_dram_parameter("hT0", [128, 256], RDT, isOutput=False)
    c0p = nc.declare_dram_parameter("c0p", [128, 256], F32, isOutput=False)
    ident32 = nc.declare_dram_parameter("ident32", [128, 128], F32, isOutput=False)
    identg = nc.declare_dram_parameter("identg", [128, 128], GDT, isOutput=False)
    ones_row = nc.declare_dram_parameter("ones_row", [1, 128], F32, isOutput=False)

    meansT_out = nc.declare_dram_parameter("meansT_out", [AC, t_steps * B], F32, isOutput=True)
    hT_out = nc.declare_dram_parameter("hT_out", [128, 256], F32, isOutput=True)
    cT_out = nc.declare_dram_parameter("cT_out", [128, 256], F32, isOutput=True)

    gx_loc = nc.dram_tensor("gx_loc", [TS, 128, 1024], GDT)
    gx_all = nc.dram_tensor("gx_all", [t_steps, 128, 1024], GDT, addr_space="Shared")

    with tile.TileContext(nc) as tc:
        # =================== phase 1: x-proj + gx (T-sharded) ==============
        with tc.tile_pool(name="persist", bufs=1) as pp:
            # persistent small tiles
            id32 = pp.tile([128, 128], F32)
            nc.sync.dma_start(id32[:], ident32[:])
            idg = pp.tile([128, 128], GDT)
            nc.sync.dma_start(idg[:], identg[:])
            ones_sb = pp.tile([1, 128], F32)
            nc.sync.dma_start(ones_sb[:], ones_row[:])
            sdup_sb = pp.tile([128, t_steps + 1], F32)
            nc.sync.dma_start(sdup_sb[:], s_dup[:])
            wm_sb = pp.tile([128, 4 * AC], RDT)
            nc.sync.dma_start(wm_sb[:], w_meanT[:])
            bm_sb = pp.tile([AC, 1], F32)
            nc.sync.dma_start(bm_sb[:], b_mean_p[:])
            biasr_sb = pp.tile([1, 4 * C], F32)
            nc.sync.dma_start(biasr_sb[:], bias_row[:])
            wrec_sb = pp.tile([128, 4, 4 * C], RDT)      # [q, k, col]
            wrec_view = w_rec.rearrange("(k q) c -> q k c", k=4)
            nc.sync.dma_start(wrec_sb[:], wrec_view)

            with tc.tile_pool(name="prep", bufs=1) as prep, \
                 tc.tile_pool(name="prep_ps", bufs=2, space="PSUM") as prep_ps, \
                 tc.tile_pool(name="gx_ps", bufs=1, space="PSUM") as gx_ps, \
                 tc.tile_pool(name="gxev", bufs=2) as gxev:
                winT_sb = prep.tile([128, H], BF16)
                nc.sync.dma_start(winT_sb[:], w_inT[:])
                binc_sb = prep.tile([128, H // 128], F32)
                nc.sync.dma_start(binc_sb[:], b_in_col[:])
                xsT_sb = prep.tile([128, ROWS], BF16)
                nc.sync.dma_start(xsT_sb[:], xsT[:])
                wih_sb = prep.tile([128, 8, 4 * C], BF16)   # [q, k, col]
                wih_view = w_ihT.rearrange("(k q) c -> q k c", k=8)
                nc.sync.dma_start(wih_sb[:], wih_view)

                # x-proj: xT[m] = relu(W_inT[:,m].T @ xsT + b_in[m])
                xT = []
                for m in range(8):
                    xm = prep.tile([128, ROWS], BF16, tag=f"xT{m}")
                    xT.append(xm)
                    for n in range(ROWS // 512):
                        ps = prep_ps.tile([128, 512], F32, tag="xp")
                        nc.tensor.matmul(ps[:], winT_sb[:, m * 128:(m + 1) * 128],
                                         xsT_sb[:, n * 512:(n + 1) * 512],
                                         start=True, stop=True)
                        nc.scalar.activation(xm[:, n * 512:(n + 1) * 512], ps[:],
                                             AF.Relu, bias=binc_sb[:, m:m + 1])

                # gx rows: 128-row chunks (= 2 timesteps)
                for r in range(RCH):
                    gp = gx_ps.tile([128, 2048], F32, tag="gxp")
                    # bias first (start=True per bank)
                    for n in range(4):
                        nc.tensor.matmul(gp[:, n * 512:(n + 1) * 512], ones_sb[:],
                                         biasr_sb[:, n * 512:(n + 1) * 512],
                                         start=True, stop=False)
                    for k in range(8):
                        lhs = xT[k][:, r * 128:(r + 1) * 128]
                        for n in range(4):
                            nc.tensor.matmul(gp[:, n * 512:(n + 1) * 512], lhs,
                                             wih_sb[:, k, n * 512:(n + 1) * 512],
                                             start=False, stop=(k == 7))
                    gb = gxev.tile([128, 2048], GDT, tag="gxbuf")
                    nc.vector.tensor_copy(gb[:, 0:1024], gp[:, 0:1024])
                    nc.scalar.copy(gb[:, 1024:2048], gp[:, 1024:2048])
                    # scatter to gx_loc[(2r, 2r+1)]: src [b, (h, g)] -> dst [b, h, g]
                    dstv = gx_loc.rearrange("t (h b) g -> t b h g", h=2)
                    for dt in range(2):
                        nc.sync.dma_start(
                            dstv[2 * r + dt],
                            gb[dt * 64:(dt + 1) * 64, :].rearrange(
                                "p (h g) -> p h g", h=2))

            # one collective: all-gather the gx slices
            if mode != "noag":
                nc.gpsimd.collective_compute(
                    "AllGather", OP.bypass,
                    replica_groups=[list(range(NCORES))],
                    ins=[gx_loc[:]], outs=[gx_all[:]],
                )
            gx_src = gx_all if mode != "noag" else gx_loc

            # =================== phase 2: recurrence ========================
            if mode == "prep":
                nc.finalize_marker = True
            with tc.tile_pool(name="rec", bufs=2) as rp, \
                 tc.tile_pool(name="gxin", bufs=8) as gxp, \
                 tc.tile_pool(name="rec_ps", bufs=2, space="PSUM") as rps, \
                 tc.tile_pool(name="aux_ps", bufs=1, space="PSUM") as aps, \
                 tc.tile_pool(name="ring", bufs=2) as ringp:

                hT_prev = rp.tile([128, 256], RDT, tag="hT")
                nc.sync.dma_start(hT_prev[:], hT0[:])
                c_prev = rp.tile([128, 256], F32, tag="c")
                nc.sync.dma_start(c_prev[:], c0p[:])

                MRING = 64
                hu_ring = None
                mring = None
                hu_last = None
                c_last = None
                for t in range(t_steps if mode != "prep" else 0):
                    if t % 4 == 0:
                        hu_ring = ringp.tile([128, 4 * 256], RDT, tag="huring")
                    if t % MRING == 0:
                        mring = ringp.tile([AC, MRING * B], F32, tag="mring")
                    gxt = gxp.tile([128, 1024], GDT, tag="gxt")
                    nc.sync.dma_start(gxt[:], gx_src[t % (t_steps if mode != "noag" else t_steps // NCORES)])

                    # next-step mask row broadcast to [128, 64] (s_{t+1})
                    s_row = gxp.tile([1, B], F32, tag="s_row")
                    nc.sync.dma_start(s_row[:], s_flat[:, (t + 1) * B:(t + 2) * B])
                    sps = aps.tile([128, B], F32, tag="sps")
                    nc.tensor.matmul(sps[:], ones_sb[:], s_row[:],
                                     start=True, stop=True)
                    s_sb = rp.tile([128, B], F32, tag="s_sb")
                    nc.scalar.copy(s_sb[:], sps[:])

                    # gates = gx + W_rec @ h  (PSUM [128,1024])
                    # per half: inject gx (start) then accumulate W_rec @ h.
                    G = rps.tile([128, 1024], F32, tag="G")
                    for half in ((0,) if mode == "onehalf" else (0, 1)):
                        tp = (0, 0) if half == 0 else (0, 64)
                        po = half * 64
                        for n in (0, 1):
                            nc.tensor.matmul(G[po:po + 64, n * 512:(n + 1) * 512],
                                             idg[po:po + 64, po:po + 64],
                                             gxt[po:po + 64, n * 512:(n + 1) * 512],
                                             start=True, stop=False,
                                             tile_position=(po, po))
                        for k in range(4):
                            lhs = hT_prev[:, k * 64:(k + 1) * 64]
                            for n in (0, 1):
                                nc.tensor.matmul(
                                    G[po:po + 64, n * 512:(n + 1) * 512], lhs,
                                    wrec_sb[:, k, half * 1024 + n * 512:
                                            half * 1024 + (n + 1) * 512],
                                    start=False, stop=(k == 3),
                                    tile_position=tp)

                    sig = rp.tile([128, 768], F32, tag="sig")
                    nc.scalar.activation(sig[:], G[:, 0:768], AF.Sigmoid)
                    tg = rp.tile([128, 256], F32, tag="tg")
                    nc.scalar.activation(tg[:], G[:, 768:1024], AF.Tanh)

                    # c' = sigf*(c*s_t) + sigi*tg
                    m2 = rp.tile([128, 256], F32, tag="m2")
                    nc.vector.scalar_tensor_tensor(m2[:], c_prev[:],
                                                   sdup_sb[:, t:t + 1],
                                                   sig[:, 256:512], OP.mult, OP.mult)
                    m3 = rp.tile([128, 256], F32, tag="m3")
                    nc.gpsimd.tensor_mul(m3[:], sig[:, 0:256], tg[:])
                    c_new = rp.tile([128, 256], F32, tag="c")
                    nc.vector.tensor_add(c_new[:], m2[:], m3[:])

                    thc = rp.tile([128, 256], F32, tag="thc")
                    nc.scalar.activation(thc[:], c_new[:], AF.Tanh)
                    hu = rp.tile([128, 256], F32, tag="hu")
                    nc.vector.tensor_mul(hu[:], thc[:], sig[:, 512:768])

                    # transpose h' -> hT chunks; dual evacuation (masked + raw)
                    slot = t % 4
                    if mode == "notr":
                        hT_new = hT_prev
                    else:
                        trp = rps.tile([128, 256], F32, tag="trp")
                        for cw in (0, 1):
                            nc.tensor.transpose(trp[:, cw * 128:(cw + 1) * 128],
                                                hu[:, cw * 128:(cw + 1) * 128],
                                                id32[:])
                        hT_new = rp.tile([128, 256], RDT, tag="hT")
                        huT = hu_ring[:, slot * 256:(slot + 1) * 256]
                        # chunk k=(half*2+cw) <- trp[cw][:, half*64:...]
                        for k, src_off in ((0, 0), (1, 128), (2, 64), (3, 192)):
                            nc.vector.tensor_mul(hT_new[:, k * 64:(k + 1) * 64],
                                                 trp[:, src_off:src_off + 64], s_sb[:])
                            nc.scalar.copy(huT[:, k * 64:(k + 1) * 64],
                                           trp[:, src_off:src_off + 64])

                    # means every 4 steps from hu_ring
                    if slot == 3 and mode not in ("nomean", "notr"):
                        mps = aps.tile([AC, 4 * B], F32, tag="mps")
                        rv = hu_ring.rearrange("p (s c) -> p s c", s=4)
                        for k in range(4):
                            nc.tensor.matmul(mps[:], wm_sb[:, k * AC:(k + 1) * AC],
                                             rv[:, :, k * 64:(k + 1) * 64],
                                             start=(k == 0), stop=(k == 3))
                        mt = (t // 4) % (MRING // 4)
                        nc.scalar.activation(mring[:, mt * 4 * B:(mt + 1) * 4 * B],
                                             mps[:], AF.Tanh, bias=bm_sb[:])
                        if mt == MRING // 4 - 1:
                            base = (t - MRING + 1) * B
                            nc.sync.dma_start(
                                meansT_out[:, base:base + MRING * B], mring[:])

                    hT_prev = hT_new
                    c_prev = c_new
                    hu_last = hu
                    c_last = c_new

                if mode != "prep":
                    hf = rp.tile([128, 256], F32, tag="hf")
                    nc.vector.tensor_copy(hf[:], hu_last[:])
                    nc.sync.dma_start(hT_out[:], hf[:])
                    nc.sync.dma_start(cT_out[:], c_last[:])

    nc.finalize()
    return nc


def _prep_inputs(xs, masks, h0, c0, W_in, b_in, W_ih, W_hh, b_ih, b_hh,
                 W_mean, b_mean, t_steps, rec_bf16, gx_bf16):
    RNP = ml_dtypes.bfloat16 if rec_bf16 else np.float32
    GNP = ml_dtypes.bfloat16 if gx_bf16 else np.float32
    TS = t_steps // NCORES

    grow = [0, 1, 3, 2]  # target gate order (i,f,o,g) -> source row block
    perm = np.array([grow[g] * C + h * 256 + cw
                     for h in (0, 1) for g in range(4) for cw in range(256)])

    com = {}
    com["w_inT"] = np.ascontiguousarray(W_in.T).astype(ml_dtypes.bfloat16)
    com["b_in_col"] = np.ascontiguousarray(
        b_in.reshape(H // 128, 128).T).astype(np.float32)
    com["w_ihT"] = np.ascontiguousarray(W_ih.T[:, perm]).astype(ml_dtypes.bfloat16)
    com["bias_row"] = (b_ih + b_hh)[perm].reshape(1, 4 * C).astype(np.float32)
    com["w_rec"] = np.ascontiguousarray(W_hh.T[:, perm]).astype(RNP)
    # w_meanT[q, 32k+a] = W_mean[a, 128k+q]
    wmt = np.ascontiguousarray(
        W_mean.T.reshape(4, 128, AC).transpose(1, 0, 2).reshape(128, 4 * AC))
    com["w_meanT"] = wmt.astype(RNP)
    com["b_mean_p"] = b_mean.reshape(AC, 1).astype(np.float32)

    s = (1.0 - masks[:t_steps, :, 0]).astype(np.float32)        # [T, B]
    s_ext = np.concatenate([s, np.ones((1, B), np.float32)], axis=0)  # [T+1, B]
    com["s_dup"] = np.ascontiguousarray(
        np.tile(s_ext.T, (2, 1))).astype(np.float32)            # [128, T+1]
    com["s_flat"] = s_ext.reshape(1, (t_steps + 1) * B).astype(np.float32)

    h0m = (h0 * s[0][:, None]).astype(np.float32)               # [B, C]
    hT0 = np.zeros((128, 256), np.float32)
    for k in range(4):
        hT0[:, k * 64:(k + 1) * 64] = h0m[:, k * 128:(k + 1) * 128].T
    com["hT0"] = hT0.astype(RNP)
    com["c0p"] = np.ascontiguousarray(
        c0.reshape(B, 2, 256).transpose(1, 0, 2).reshape(128, 256)).astype(np.float32)
    com["ident32"] = np.eye(128, dtype=np.float32)
    com["identg"] = np.eye(128, dtype=GNP)
    com["ones_row"] = np.ones((1, 128), np.float32)

    in_maps = []
    for kcore in range(NCORES):
        m = dict(com)
        sl = xs[kcore * TS:(kcore + 1) * TS].reshape(TS * B, OB)
        m["xsT"] = np.ascontiguousarray(sl.T).astype(ml_dtypes.bfloat16)
        in_maps.append(m)
    return in_maps


def _unpack(res, t_steps, log_std_param):
    meansT = res["meansT_out"]                                  # [AC, T*B]
    means = np.ascontiguousarray(
        meansT.reshape(AC, t_steps, B).transpose(1, 2, 0)).astype(np.float32)
    hT = np.ascontiguousarray(
        res["hT_out"].reshape(2, B, 256).transpose(1, 0, 2).reshape(B, C))
    cT = np.ascontiguousarray(
        res["cT_out"].reshape(2, B, 256).transpose(1, 0, 2).reshape(B, C))
    log_std = np.broadcast_to(
        np.asarray(log_std_param, np.float32), means.shape).copy()
    return means, log_std, hT, cT


def kernel(xs, masks, h0, c0, W_in, b_in, W_ih, W_hh, b_ih, b_hh,
           W_mean, b_mean, log_std_param, t_steps=T,
           rec_bf16=REC_BF16, gx_bf16=GX_BF16, trace=False, tmpdir=None,
           mode="full"):
    xs = np.asarray(xs, np.float32)
    masks = np.asarray(masks, np.float32)
    key = (t_steps, rec_bf16, gx_bf16, mode)
    if key not in _cache:
        _cache[key] = build(t_steps, rec_bf16, gx_bf16, mode)
    nc = _cache[key]
    in_maps = _prep_inputs(xs, masks, np.asarray(h0, np.float32),
                           np.asarray(c0, np.float32), W_in, b_in, W_ih, W_hh,
                           b_ih, b_hh, W_mean, b_mean, t_steps,
                           rec_bf16, gx_bf16)
    kw = {}
    if trace:
        kw = dict(trace=True, tmpdir=tmpdir)
    r = run_bass_kernel_spmd(nc, in_maps, core_ids=list(range(NCORES)), **kw)
    out = _unpack(r.results[0], t_steps, log_std_param)
    kernel.last_exec_ns = r.exec_time_ns
    return out


kernel.last_exec_ns = None
